# revision 1
# baseline (speedup 1.0000x reference)
"""S[b] = X[b] @ M @ Y[b]^T, data-parallel over BS across 8 NeuronCores.

BS=16, X_LEN=Y_LEN=H=1024.  Each core owns 2 batches: computes
XM = X_b @ M (M replicated), then S_b = XM @ Y_b^T.
"""
import numpy as np

BS, X_LEN, Y_LEN, H = 16, 1024, 1024, 1024
N_CORES = 8


def kernel(X: np.ndarray, Y: np.ndarray, M: np.ndarray) -> np.ndarray:
    import jax
    import jax.numpy as jnp

    devs = jax.devices()[:N_CORES]
    per = BS // N_CORES  # 2 batches per core

    Xs = np.asarray(X, np.float32).reshape(N_CORES, per, X_LEN, H)
    Ys = np.asarray(Y, np.float32).reshape(N_CORES, per, Y_LEN, H)
    Mf = np.asarray(M, np.float32)

    @jax.pmap
    def _shard(x, y, m):
        xm = jnp.einsum("bxh,hk->bxk", x, m,
                        preferred_element_type=jnp.float32)
        return jnp.einsum("bxk,byk->bxy", xm, y,
                          preferred_element_type=jnp.float32)

    Mrep = np.broadcast_to(Mf, (N_CORES, H, H))
    out = _shard(
        jax.device_put_sharded(list(Xs), devs),
        jax.device_put_sharded(list(Ys), devs),
        jax.device_put_sharded(list(Mrep), devs),
    )
    return np.asarray(out).reshape(BS, X_LEN, Y_LEN).astype(np.float32)
